# revision 77
# baseline (speedup 1.0000x reference)
"""Trainium2 kernel for nn_AttentionRotationBlock — full on-device pipeline.

Sharding (8 cores): core c handles batch b=c//4 with group rank r=c%4.
  - rmsnorm1 over the full batch is computed redundantly per core (cheap,
    avoids any gather), qkv is head-parallel: core computes q,k,v for its
    4 heads over all 2048 tokens of its batch.
  - causal attention for those 4 heads is fully local and perfectly
    balanced across cores; softmax runs in scores-transposed layout
    ([keys, queries]) with the denominator folded into the P@V matmul via
    a ones-column appended to V.
  - o-projection partials (own 256 hidden dims) are AllReduce-added (bf16)
    within each 4-core batch group; the tail (residual + rmsnorm2 + three
    rotation-GEMM/silu passes) is token-parallel on the core's own 512
    tokens.
All matmuls run in bf16 (fp32 PSUM accumulate).  Falls back to an exact
fp32 numpy path if the device path fails.
"""

import sys

import numpy as np

B, T, D, H, HD, NPASS = 2, 2048, 1024, 16, 64, 3
NCORES, R = 8, 4
TPC = T // R           # 512 own tokens per core
NKT = D // 128         # 8 feature tiles
NKB = T // 128         # 16 key blocks
NQC = 4                # query chunks of 512
QCW = T // NQC
HPC = H // R           # 4 heads per core
EPS = float(np.finfo(np.float32).eps)
GROUPS = [[0, 1, 2, 3], [4, 5, 6, 7]]
DEBUG = False


# ----------------------------------------------------------------------
# host reference pieces (fallback + input prep)
# ----------------------------------------------------------------------

def _rmsnorm(x, w):
    ms = np.mean(x * x, axis=-1, keepdims=True)
    return x * (1.0 / np.sqrt(ms + EPS)) * w


def _giv_mats(angles, pi, pj, gate):
    """Dense [D,D] G_p with gate folded in; rotated = r @ G_p."""
    mats = []
    for p in range(NPASS):
        G = np.eye(D, dtype=np.float64)
        ca = np.cos(angles[p].astype(np.float64))
        sa = np.sin(angles[p].astype(np.float64))
        ii = pi[p].astype(np.int64)
        jj = pj[p].astype(np.int64)
        G[ii, ii] = ca
        G[jj, ii] = -sa
        G[ii, jj] = sa
        G[jj, jj] = ca
        G = G * gate[p].astype(np.float64)[None, :]
        mats.append(G.astype(np.float32))
    return mats


def _host_full(x, scale_gamma, scale_beta, qkv_w, o_w, norm1_w, norm2_w,
               angles, gate, bias, pi, pj):
    xf = np.asarray(x, np.float32)
    h = _rmsnorm(xf, norm1_w) * scale_gamma + scale_beta
    qkv = (h.reshape(B * T, D) @ qkv_w.T).reshape(B, T, 3, H, HD)
    q = np.moveaxis(qkv[:, :, 0], 1, 2)
    k = np.moveaxis(qkv[:, :, 1], 1, 2)
    v = np.moveaxis(qkv[:, :, 2], 1, 2)
    scale = 1.0 / np.sqrt(HD)
    causal = np.tril(np.ones((T, T), bool))
    out = np.empty((B, H, T, HD), np.float32)
    for b in range(B):
        for hh in range(H):
            s = (q[b, hh] @ k[b, hh].T) * scale
            s = np.where(causal, s, -np.inf).astype(np.float32)
            s -= s.max(axis=-1, keepdims=True)
            e = np.exp(s)
            out[b, hh] = (e / e.sum(axis=-1, keepdims=True)) @ v[b, hh]
    ao = np.swapaxes(out, 1, 2).reshape(B, T, D)
    x2 = xf + ao @ o_w.T
    h2 = _rmsnorm(x2, norm2_w) * scale_gamma + scale_beta
    r = h2.reshape(B * T, D)
    for p, G in enumerate(_giv_mats(angles, pi, pj, gate)):
        r = r @ G + bias[p][None, :]
        r = r * (1.0 / (1.0 + np.exp(-r)))
    return (x2 + (r.reshape(B, T, D) - h2)).astype(np.float32)


# ----------------------------------------------------------------------
# walrus workaround: split multi-wait instructions (this container's
# walrus accepts only one embedded sync wait per instruction)
# ----------------------------------------------------------------------

def _split_multi_waits(nc, mybir, max_waits=1):
    n = 0
    for f in nc.m.functions:
        for bb in f.blocks:
            out = []
            for inst in bb.instructions:
                si = inst.sync_info
                if si is not None and si.on_wait and len(si.on_wait) > max_waits:
                    waits = list(si.on_wait)
                    for kk, w in enumerate(waits[:-max_waits]):
                        ev = mybir.InstEventSemaphore(
                            name=f"{inst.name}_xw{kk}",
                            engine=inst.engine,
                            ins=[],
                            outs=[],
                            sync_info=mybir.SyncInfo(on_wait=[w], on_update=[]),
                        )
                        out.append(ev)
                        n += 1
                    inst.sync_info = mybir.SyncInfo(
                        on_wait=waits[-max_waits:], on_update=list(si.on_update)
                    )
                out.append(inst)
            bb.instructions = out
    return n


# ----------------------------------------------------------------------
# device kernel
# ----------------------------------------------------------------------

def _build_nc():
    import concourse.bass as bass
    import concourse.mybir as mybir
    import concourse.tile as tile

    f32 = mybir.dt.float32
    bf16 = mybir.dt.bfloat16
    AF = mybir.ActivationFunctionType
    Alu = mybir.AluOpType

    nc = bass.Bass(num_devices=NCORES)

    xtb = nc.dram_tensor("xtb", [NKT, 128, T], bf16, kind="ExternalInput")
    xto = nc.dram_tensor("xto", [NKT, 128, TPC], f32, kind="ExternalInput")
    wqkv = nc.dram_tensor("wqkv", [NKT, 128, 768], bf16, kind="ExternalInput")
    wow = nc.dram_tensor("wow", [2, 128, D], bf16, kind="ExternalInput")
    gmat = nc.dram_tensor("gmat", [NPASS, NKT, 128, D], bf16,
                          kind="ExternalInput")
    geff1 = nc.dram_tensor("geff1", [128, NKT], f32, kind="ExternalInput")
    geff2 = nc.dram_tensor("geff2", [128, NKT], f32, kind="ExternalInput")
    betav = nc.dram_tensor("betav", [128, NKT], f32, kind="ExternalInput")
    biasr = nc.dram_tensor("biasr", [128, NKT, NPASS], f32,
                           kind="ExternalInput")
    trim = nc.dram_tensor("trim", [128, 2, 128], bf16, kind="ExternalInput")
    cqk = nc.dram_tensor("cqk", [128, 4], f32, kind="ExternalInput")
    cvb = nc.dram_tensor("cvb", [128, 256], bf16, kind="ExternalInput")
    rofs = nc.dram_tensor("rofs", [1, 2], mybir.dt.uint32,
                          kind="ExternalInput")
    yt = nc.dram_tensor("yt", [NKT, 128, TPC], f32, kind="ExternalOutput")
    if DEBUG:
        dbg_qk = nc.dram_tensor("dbg_qk", [128, 4, T], bf16,
                                kind="ExternalOutput")
        dbg_v = nc.dram_tensor("dbg_v", [128, NKB, HPC, HD + 1], bf16,
                               kind="ExternalOutput")
        dbg_at = nc.dram_tensor("dbg_at", [128, 2, T], bf16,
                                kind="ExternalOutput")
        dbg_rs = nc.dram_tensor("dbg_rs", [128, NKT, T], bf16,
                                kind="ExternalOutput")

    with tile.TileContext(nc) as tc:
        with (
            tc.tile_pool(name="big", bufs=1) as big,
            tc.tile_pool(name="med", bufs=2) as med,
            tc.tile_pool(name="small", bufs=1) as small,
            tc.tile_pool(name="ps", bufs=2, space="PSUM") as ps,
            tc.tile_pool(name="pso", bufs=2, space="PSUM") as pso,
            tc.tile_pool(name="dram", bufs=1, space="DRAM") as dram,
        ):
            # ---------------- constants / small loads ----------------
            ones_b = small.tile([128, 1], bf16, tag="ones_b")
            nc.vector.memset(ones_b[:, :], 1.0)
            # ones rows for K=1 partition-broadcast matmuls
            onr_b = small.tile([1, 128], bf16, tag="onr_b")
            nc.vector.memset(onr_b[:, :], 1.0)
            onr_f = small.tile([1, 128], f32, tag="onr_f")
            nc.vector.memset(onr_f[:, :], 1.0)
            eps_t = small.tile([1, 1], f32, tag="eps_t")
            nc.vector.memset(eps_t[:, :], EPS)
            g2_t = small.tile([128, NKT], f32, tag="g2_t")
            nc.sync.dma_start(out=g2_t[:, :], in_=geff2[:, :])
            be_t = small.tile([128, NKT], f32, tag="be_t")
            nc.sync.dma_start(out=be_t[:, :], in_=betav[:, :])
            br_t = small.tile([128, NKT, NPASS], f32, tag="br_t")
            nc.sync.dma_start(out=br_t[:, :, :], in_=biasr[:, :, :])
            tri_t = small.tile([128, 2, 128], bf16, tag="tri_t")
            nc.sync.dma_start(out=tri_t[:, :, :], in_=trim[:, :, :])

            # ---------------- phase A: load x^T; rmsnorm1 stats --------------
            # geff1 is folded into wqkv on the host and rstd commutes through
            # the qkv matmul as a per-token scalar, so qkv runs on raw x and
            # the norm statistics overlap it; rstd/beta are applied at PSUM
            # evacuation time.
            xt = big.tile([128, NKT, T], bf16, tag="xt")
            wq_t = big.tile([128, NKT, 768], bf16, tag="wq", bufs=2)
            nc.sync.dma_start(out=wq_t[:, :, :],
                              in_=wqkv[:, :, :].rearrange("k p t -> p k t"))
            cqk_t = small.tile([128, 4], f32, tag="cqk_t")
            nc.sync.dma_start(out=cqk_t[:, :], in_=cqk[:, :])
            cvb_t = small.tile([128, 256], bf16, tag="cvb_t")
            nc.sync.dma_start(out=cvb_t[:, :], in_=cvb[:, :])

            # per-chunk loads + norm stats are emitted inside the chunk loop
            # below so chunk 0's collective path is not queued behind the
            # other chunks' stats.  rstd = exp(-0.5 * ln(ssq/D + eps))
            # (one ACT table set; no sqrt table, no DVE reciprocal)
            lnms = small.tile([1, T], f32, tag="lnms")
            rstd = small.tile([1, T], bf16, tag="rstd")
            rstdB = small.tile([128, T], bf16, tag="rstdB")
            rtp_b = small.tile([128, NKB], f32, tag="rtp_b")

            # ---------------- phases B+C+D interleaved per query chunk -------
            # qkv for chunk qc -> attention chunk qc -> o-proj partial ->
            # chunked AllReduce (overlaps later chunks)
            wo_t = big.tile([128, 2, D], bf16, tag="wo")
            nc.sync.dma_start(out=wo_t[:, :, :],
                              in_=wow[:, :, :].rearrange("k p t -> p k t"))
            qkT = big.tile([128, 4, T], bf16, tag="qkT")
            v_sb = big.tile([128, NKB, HPC, HD + 1], bf16, tag="v_sb")
            nc.vector.memset(v_sb[:, :, :, HD:HD + 1], 1.0)
            attnT = big.tile([128, 2, T], bf16, tag="attnT")
            arin = [dram.tile([NKT, 128, QCW], bf16, name=f"arin{q}")
                    for q in range(NQC)]
            arout = [dram.tile([NKT, 128, QCW], bf16, name=f"arout{q}")
                    for q in range(NQC)]
            for qc in range(NQC):
                csl = slice(qc * QCW, (qc + 1) * QCW)
                # --- x loads + norm1 stats for this chunk ---
                for k in range(NKT):
                    nc.sync.dma_start(out=xt[:, k, csl], in_=xtb[k, :, csl])
                spool = pso if qc == 0 else ps
                stag = "out" if qc == 0 else "mm"
                ssq = spool.tile([1, QCW], f32, tag=stag, name=f"ssq{qc}")
                for k in range(NKT):
                    sq = med.tile([128, QCW], bf16, tag="sq")
                    nc.vector.tensor_mul(out=sq[:, :], in0=xt[:, k, csl],
                                         in1=xt[:, k, csl])
                    nc.tensor.matmul(ssq[:, :], ones_b[:, :], sq[:, :],
                                     start=(k == 0), stop=(k == NKT - 1))
                nc.scalar.activation(out=lnms[:, csl], in_=ssq[:, :],
                                     func=AF.Ln, scale=1.0 / D,
                                     bias=eps_t[:, :])
                nc.scalar.activation(out=rstd[:, csl], in_=lnms[:, csl],
                                     func=AF.Exp, scale=-0.5)
                bp = spool.tile([128, 2, QCW], f32, tag=stag,
                                name=f"bp{qc}")
                nc.tensor.matmul(bp[:, 0, :], onr_b[:, :], rstd[:, csl],
                                 start=True, stop=True)
                nc.vector.tensor_copy(out=rstdB[:, csl], in_=bp[:, 0, :])
                # rstd transposed to [token-partition, block] for v evac
                for i, tt in enumerate(range(4 * qc, 4 * qc + 4)):
                    nc.tensor.matmul(bp[:, 1, i:i + 1],
                                     rstd[:, tt * 128:(tt + 1) * 128],
                                     onr_b[:, 0:1],
                                     start=(i == 0), stop=(i == 3))
                nc.vector.tensor_copy(out=rtp_b[:, 4 * qc:4 * qc + 4],
                                      in_=bp[:, 1, 0:4])
                # --- qkv for this token chunk (m order 0,2,1,3 so head-pair
                # 0's q AND k land in the first slot wave) ---
                for m in (0, 2, 1, 3):  # 0,1: q head-pairs; 2,3: k pairs
                    qp = ps.tile([128, 2, QCW], f32, tag="mm",
                                 name=f"qk{qc}{m}")
                    for k in range(NKT):
                        nc.tensor.matmul(qp[:, 0, :],
                                         wq_t[:, k, m * 128:(m + 1) * 128],
                                         xt[:, k, csl],
                                         start=(k == 0), stop=(k == NKT - 1))
                    # qkT = rstd * raw + cqk[m]
                    nc.vector.tensor_mul(out=qkT[:, m, csl], in0=qp[:, 0, :],
                                         in1=rstdB[:, csl])
                    nc.vector.tensor_scalar(
                        out=qkT[:, m, csl], in0=qkT[:, m, csl],
                        scalar1=cqk_t[:, m:m + 1], scalar2=None, op0=Alu.add)
                # v blocks descending: the descending-j chain uses v[jmax]
                # first
                for tt in range(4 * qc + 3, 4 * qc - 1, -1):
                    vp = ps.tile([128, 2, QCW], f32, tag="mm", name=f"v{tt}")
                    for k in range(NKT):
                        nc.tensor.matmul(vp[:, 0, 0:256],
                                         xt[:, k, tt * 128:(tt + 1) * 128],
                                         wq_t[:, k, 512:768],
                                         start=(k == 0), stop=(k == NKT - 1))
                    nc.vector.tensor_scalar(
                        out=v_sb[:, tt, :, 0:HD],
                        in0=vp[:, 0, 0:256].rearrange("p (h d) -> p h d",
                                                      h=HPC),
                        scalar1=rtp_b[:, tt:tt + 1], scalar2=None,
                        op0=Alu.mult)
                    nc.vector.tensor_add(
                        out=v_sb[:, tt, :, 0:HD], in0=v_sb[:, tt, :, 0:HD],
                        in1=cvb_t[:, :].rearrange("p (h d) -> p h d", h=HPC))
                # --- attention for this chunk (both head-pair chains first,
                # then all divisions, so hp1's chain overlaps hp0's) ---
                outps = []
                jmax = 4 * qc + 3
                for hp in range(2):
                    outp = pso.tile([65, 2, QCW], f32, tag="out",
                                    name=f"o{hp}{qc}")
                    outps.append(outp)
                    for j in range(jmax, -1, -1):   # descending j
                        cs = max(0, (j - 4 * qc) * 128)
                        scp = ps.tile([128, 2, QCW], f32, tag="mm",
                                      name=f"s{hp}{qc}{j}")
                        for h01 in range(2):
                            pr = slice(64 * h01, 64 * h01 + 64)
                            nc.tensor.matmul(
                                scp[:, h01, cs:QCW],
                                qkT[pr, 2 + hp, j * 128:(j + 1) * 128],
                                qkT[pr, hp, qc * QCW + cs:(qc + 1) * QCW],
                                start=True, stop=True)
                        pt = med.tile([128, 2, QCW], bf16, tag="pt", bufs=3)
                        nc.scalar.activation(out=pt[:, :, cs:QCW],
                                             in_=scp[:, :, cs:QCW],
                                             func=AF.Exp, scale=0.125)
                        if j >= 4 * qc:
                            nc.vector.tensor_mul(out=pt[:, :, cs:cs + 128],
                                                 in0=pt[:, :, cs:cs + 128],
                                                 in1=tri_t[:, :, :])
                        for h01 in range(2):
                            h = 2 * hp + h01
                            nc.tensor.matmul(
                                outp[:, h01, cs:QCW],
                                v_sb[:, j, h, :],
                                pt[:, h01, cs:QCW],
                                start=(j == jmax), stop=(j == 0))
                # divisions, steps interleaved across the 4 heads
                dens, dbps, recs = [], [], []
                for hp in range(2):
                    for h01 in range(2):
                        den = med.tile([1, QCW], bf16, tag="den", bufs=4,
                                       name=f"dn{hp}{qc}{h01}")
                        nc.vector.tensor_copy(out=den[:, :],
                                              in_=outps[hp][64:65, h01, :])
                        dens.append(den)
                for hp in range(2):
                    for h01 in range(2):
                        dbp = ps.tile([128, 2, QCW], f32, tag="mm",
                                      name=f"db{hp}{qc}{h01}")
                        nc.tensor.matmul(dbp[0:64, 0, :], onr_b[:, 0:64],
                                         dens[2 * hp + h01][:, :],
                                         start=True, stop=True)
                        dbps.append(dbp)
                for hp in range(2):
                    for h01 in range(2):
                        rec = med.tile([64, QCW], f32, tag="rec", bufs=4,
                                       name=f"rc{hp}{qc}{h01}")
                        nc.vector.reciprocal(out=rec[:, :],
                                             in_=dbps[2 * hp + h01][0:64, 0, :])
                        recs.append(rec)
                for hp in range(2):
                    for h01 in range(2):
                        h = 2 * hp + h01
                        pr = slice(64 * (h % 2), 64 * (h % 2) + 64)
                        nc.vector.tensor_mul(
                            out=attnT[pr, h // 2, qc * QCW:(qc + 1) * QCW],
                            in0=outps[hp][0:64, h01, :],
                            in1=recs[2 * hp + h01][:, :])
                # o-proj partial for this query chunk, then its AllReduce
                dst = med.tile([128, NKT, QCW], bf16, tag="dst")
                for m in range(NKT):
                    dp = ps.tile([128, 2, QCW], f32, tag="mm",
                                 name=f"d{qc}{m}")
                    for k in range(2):
                        nc.tensor.matmul(dp[:, 0, :],
                                         wo_t[:, k, m * 128:(m + 1) * 128],
                                         attnT[:, k, qc * QCW:(qc + 1) * QCW],
                                         start=(k == 0), stop=(k == 1))
                    if m % 2 == 0 and qc < NQC - 1:
                        nc.vector.tensor_copy(out=dst[:, m, :],
                                              in_=dp[:, 0, :])
                    else:
                        # last chunk: all evacuations on ACT to keep DVE
                        # free for the division chain
                        nc.scalar.activation(out=dst[:, m, :],
                                             in_=dp[:, 0, :], func=AF.Copy)
                    nc.sync.dma_start(out=arin[qc][m, :, :],
                                      in_=dst[:, m, :])
                nc.gpsimd.collective_compute(
                    "AllReduce", Alu.add,
                    replica_groups=GROUPS,
                    ins=[arin[qc].opt()], outs=[arout[qc].opt()],
                )

            # ---------------- phase E: tail on own 512 tokens ----------------
            # the AllReduce result is identical across the group; each core
            # reads it all (reusing xt's slot) and selects its own 512-token
            # slice with a runtime register offset (rofs, host-supplied).
            rsb = big.tile([128, NKT, T], bf16, tag="xt", name="rsb")
            for q in range(NQC):
                # gpsimd (SWDGE) so these collective-gated reads never stall
                # the sync queue that carries the arin bounce writes; per-k
                # so the x2 adds pipeline with the last chunk's read
                for k in range(NKT):
                    nc.gpsimd.dma_start(
                        out=rsb[:, k, q * QCW:(q + 1) * QCW],
                        in_=arout[q][k, :, :])
            if DEBUG:
                nc.sync.dma_start(out=dbg_qk[:, :, :], in_=qkT[:, :, :])
                nc.sync.dma_start(out=dbg_v[:, :, :, :], in_=v_sb[:, :, :, :])
                nc.sync.dma_start(out=dbg_at[:, :, :], in_=attnT[:, :, :])
                nc.sync.dma_start(out=dbg_rs[:, :, :], in_=rsb[:, :, :])
            rreg = nc.vector.alloc_register("rofs_reg")
            nc.vector.reg_load(rreg, rofs[0:1, 0:1])
            roff = nc.vector.snap(rreg, donate=True, min_val=0,
                                  max_val=T - TPC)
            x2 = big.tile([128, NKT, TPC], f32, tag="x2")
            x2b = big.tile([128, NKT, TPC], bf16, tag="h2", name="x2b")
            # rmsnorm2 stats (same ln/exp trick); the x2b half-0 casts are
            # emitted first so rotation pass 0 (which needs only half 0)
            # starts asap, then stats follow
            lnms2 = small.tile([1, TPC], f32, tag="lnms2")
            rstd2 = small.tile([1, TPC], bf16, tag="rstd2")
            rstd2B = small.tile([128, TPC], bf16, tag="rstd2B")
            bp2 = pso.tile([128, 2, QCW], f32, tag="out", name="bp2")
            h0 = slice(0, TPC // 2)
            h1 = slice(TPC // 2, TPC)
            for k in range(NKT):
                xo_k = med.tile([128, TPC], f32, tag="xo")
                nc.sync.dma_start(out=xo_k[:, :], in_=xto[k, :, :])
                nc.vector.tensor_add(out=x2[:, k, :], in0=xo_k[:, :],
                                     in1=rsb[:, k, bass.ds(roff, TPC)])
                nc.vector.tensor_copy(out=x2b[:, k, h0], in_=x2[:, k, h0])
            ssq2 = pso.tile([1, TPC], f32, tag="out", name="ssq2")
            for k in range(NKT):
                nc.vector.tensor_copy(out=x2b[:, k, h1], in_=x2[:, k, h1])
                sq2 = med.tile([128, TPC], bf16, tag="sq2")
                nc.scalar.activation(out=sq2[:, :], in_=x2[:, k, :],
                                     func=AF.Square)
                nc.tensor.matmul(ssq2[:, :], ones_b[:, :], sq2[:, :],
                                 start=(k == 0), stop=(k == NKT - 1))
            nc.scalar.activation(out=lnms2[:, :], in_=ssq2[:, :],
                                 func=AF.Ln, scale=1.0 / D, bias=eps_t[:, :])
            nc.scalar.activation(out=rstd2[:, :], in_=lnms2[:, :],
                                 func=AF.Exp, scale=-0.5)
            nc.tensor.matmul(bp2[:, 0, :], onr_b[:, :], rstd2[:, :],
                             start=True, stop=True)
            nc.vector.tensor_copy(out=rstd2B[:, :], in_=bp2[:, 0, :])
            # geff2 is folded into G_0 on the host (plus a beta@G_0 bias
            # shift) and rstd2 commutes through the pass-0 matmul as a
            # per-token scalar: pass 0 runs on raw bf16 x2 (cast above, no
            # wait on the norm2 stats); rstd2 is applied at PSUM evacuation.

            # rotation passes (gate folded into G; bias separate).  G for
            # pass 0/2 lands in the freed wq slots, pass 1 in the freed qkT
            # slot; all three loads are emitted early so they prefetch
            # behind the AllReduce chain.
            gh01 = big.tile([128, 4, D], bf16, tag="wq", bufs=2, name="gh01")
            nc.sync.dma_start(out=gh01[:, :, :],
                              in_=gmat[0, 0:4, :, :].rearrange("k p t -> p k t"))
            gh00 = big.tile([128, 4, D], bf16, tag="wq", bufs=2, name="gh00")
            nc.sync.dma_start(out=gh00[:, :, :],
                              in_=gmat[0, 4:8, :, :].rearrange("k p t -> p k t"))
            gh1 = big.tile([128, NKT, D], bf16, tag="qkT", name="gh1")
            nc.sync.dma_start(out=gh1[:, :, :],
                              in_=gmat[1, :, :, :].rearrange("k p t -> p k t"))
            gslices = {
                0: [gh01[:, :, :], gh00[:, :, :]],
                1: [gh1[:, 0:4, :], gh1[:, 4:8, :]],
            }
            rb0 = big.tile([128, NKT, TPC], bf16, tag="rb0")
            rb1 = big.tile([128, NKT, TPC], bf16, tag="rb1")
            cur = x2b
            for p3 in range(NPASS):
                last = p3 == NPASS - 1
                nxt = None if last else (rb0 if p3 == 0 else rb1)
                if p3 < 2:
                    gh = gslices[p3]
                else:
                    gh = []
                    for half in range(2):
                        gt_ = big.tile([128, 4, D], bf16, tag="wq", bufs=2,
                                       name=f"gh2{half}")
                        nc.sync.dma_start(
                            out=gt_[:, :, :],
                            in_=gmat[2, 4 * half:4 * half + 4, :, :]
                            .rearrange("k p t -> p k t"))
                        gh.append(gt_[:, :, :])
                # token-halved so consecutive passes pipeline: pass p+1 on
                # half 0 starts once pass p finished half 0
                for th in range(2):
                    tsl = slice(th * (TPC // 2), (th + 1) * (TPC // 2))
                    for m in range(NKT):
                        rp = ps.tile([128, 2, QCW], f32, tag="mm",
                                     name=f"r{p3}{m}{th}")
                        for k in range(NKT):
                            nc.tensor.matmul(
                                rp[:, 0, 0:TPC // 2],
                                gh[k // 4][:, k % 4,
                                           m * 128:(m + 1) * 128],
                                cur[:, k, tsl],
                                start=(k == 0), stop=(k == NKT - 1))
                        if p3 == 0:
                            # apply the commuted rstd2 before the silu
                            rn = med.tile([128, TPC // 2], bf16, tag="rn")
                            nc.vector.tensor_mul(out=rn[:, :],
                                                 in0=rp[:, 0, 0:TPC // 2],
                                                 in1=rstd2B[:, tsl])
                            nc.scalar.activation(
                                out=nxt[:, m, tsl], in_=rn[:, :],
                                func=AF.Silu, bias=br_t[:, m, p3:p3 + 1])
                        elif not last:
                            nc.scalar.activation(
                                out=nxt[:, m, tsl], in_=rp[:, 0, 0:TPC // 2],
                                func=AF.Silu, bias=br_t[:, m, p3:p3 + 1])
                        else:
                            # final: y_m = (x2_m - h2_m) + silu(rp); the
                            # subtraction was folded into x2 during pass 1
                            rl = med.tile([128, TPC // 2], f32, tag="rl")
                            nc.scalar.activation(
                                out=rl[:, :], in_=rp[:, 0, 0:TPC // 2],
                                func=AF.Silu, bias=br_t[:, m, p3:p3 + 1])
                            nc.vector.tensor_add(out=x2[:, m, tsl],
                                                 in0=x2[:, m, tsl],
                                                 in1=rl[:, :])
                    if last:
                        nc.sync.dma_start(
                            out=yt[:, :, tsl].rearrange("k p t -> p k t"),
                            in_=x2[:, :, tsl])
                if p3 == 0:
                    # h2 and the (x2 - h2) fold, overlapped with pass 1
                    for k in range(NKT):
                        h2k = med.tile([128, TPC], bf16, tag="h2k", bufs=2,
                                       name=f"h2k{k}")
                        nc.vector.tensor_mul(out=h2k[:, :],
                                             in0=x2b[:, k, :],
                                             in1=rstd2B[:, :])
                        nc.vector.tensor_scalar(
                            out=h2k[:, :], in0=h2k[:, :],
                            scalar1=g2_t[:, k:k + 1],
                            scalar2=be_t[:, k:k + 1],
                            op0=Alu.mult, op1=Alu.add)
                        nc.vector.tensor_sub(out=x2[:, k, :],
                                             in0=x2[:, k, :],
                                             in1=h2k[:, :])
                cur = nxt
    return nc


def _prep_inputs(x, scale_gamma, scale_beta, qkv_w, o_w, norm1_w, norm2_w,
                 angles, gate, bias, pi, pj):
    import ml_dtypes
    bf = ml_dtypes.bfloat16
    x = np.asarray(x, np.float32)
    gmats = _giv_mats(np.asarray(angles), np.asarray(pi), np.asarray(pj),
                      np.asarray(gate))
    g1 = (np.asarray(norm1_w) * np.asarray(scale_gamma)).astype(np.float32)
    g2 = (np.asarray(norm2_w) * np.asarray(scale_gamma)).astype(np.float32)
    beta = np.asarray(scale_beta, np.float32)
    # fold geff2 into G_0 (rotation pass 0 runs on x2n = x2*rstd2); the
    # beta part of h2 projects into the pass-0 bias
    bias = np.asarray(bias, np.float32).copy()
    bias[0] = bias[0] + beta @ gmats[0]
    gmats[0] = g2[:, None] * gmats[0]
    gm = np.stack([g.reshape(NKT, 128, D) for g in gmats]).astype(bf)
    g1_t = np.ascontiguousarray(g1.reshape(NKT, 128).T)
    g2_t = np.ascontiguousarray(g2.reshape(NKT, 128).T)
    be_t = np.ascontiguousarray(beta.reshape(NKT, 128).T)
    br = bias.reshape(NPASS, NKT, 128)
    br_t = np.ascontiguousarray(np.transpose(br, (2, 1, 0)))
    tri = np.tril(np.ones((128, 128), np.float32)).T  # [kl, ql] kl<=ql
    tri2 = np.ascontiguousarray(
        np.broadcast_to(tri[:, None, :], (128, 2, 128))).astype(bf)
    qkv_wT = np.asarray(qkv_w, np.float32).T  # [D, 3D]
    o_wT = np.asarray(o_w, np.float32).T      # [D(d), D(j)]

    in_maps = []
    for c in range(NCORES):
        b, r = divmod(c, R)
        xT = np.ascontiguousarray(x[b].T)          # [D, T]
        hsel = np.r_[256 * r:256 * r + 256]
        wsel = np.concatenate([qkv_wT[:, hsel], qkv_wT[:, 1024 + hsel],
                               qkv_wT[:, 2048 + hsel]], axis=1)  # [D, 768]
        # fold geff1 into the weight rows; beta projects to a const vector
        wfold = wsel * g1[:, None]
        cvec = beta @ wsel                          # [768]
        cqk_m = np.ascontiguousarray(
            cvec[:512].reshape(4, 128).T).astype(np.float32)   # [128, 4]
        cvb_m = np.ascontiguousarray(
            np.broadcast_to(cvec[512:768], (128, 256))).astype(bf)
        m = dict(
            xtb=xT.reshape(NKT, 128, T).astype(bf),
            xto=np.ascontiguousarray(
                xT[:, r * TPC:(r + 1) * TPC]).reshape(NKT, 128, TPC),
            wqkv=np.ascontiguousarray(wfold).reshape(NKT, 128, 768).astype(bf),
            wow=np.ascontiguousarray(
                o_wT[256 * r:256 * r + 256]).reshape(2, 128, D).astype(bf),
            gmat=gm,
            geff1=g1_t, geff2=g2_t, betav=be_t, biasr=br_t,
            trim=tri2,
            cqk=cqk_m, cvb=cvb_m,
            rofs=np.array([[r * TPC, r * TPC + TPC // 2]], np.uint32),
        )
        in_maps.append(m)
    return in_maps


_NC_CACHE = [None]


def _device_run(in_maps):
    from concourse import bass_utils
    import concourse.mybir as mybir

    if _NC_CACHE[0] is None:
        nc = _build_nc()
        _split_multi_waits(nc, mybir)
        _NC_CACHE[0] = nc
    nc = _NC_CACHE[0]
    res = bass_utils.run_bass_kernel_spmd(nc, in_maps,
                                          core_ids=list(range(NCORES)))
    y = np.empty((B, T, D), np.float32)
    for c in range(NCORES):
        b, r = divmod(c, R)
        yT = res.results[c]["yt"].reshape(D, TPC)
        y[b, r * TPC:(r + 1) * TPC] = yT.T
    return y


def kernel(x, scale_gamma, scale_beta, qkv_w, o_w, norm1_w, norm2_w,
           angles, gate, bias, pi, pj):
    args = dict(x=x, scale_gamma=scale_gamma, scale_beta=scale_beta,
                qkv_w=qkv_w, o_w=o_w, norm1_w=norm1_w, norm2_w=norm2_w,
                angles=angles, gate=gate, bias=bias, pi=pi, pj=pj)
    try:
        in_maps = _prep_inputs(**args)
        return _device_run(in_maps)
    except Exception as e:
        print(f"device path failed ({type(e).__name__}: {e}); "
              "using host fallback", file=sys.stderr)
        import traceback
        traceback.print_exc()
        return _host_full(**args)


# revision 78
# speedup vs baseline: 1.0016x; 1.0016x over previous
"""Trainium2 kernel for nn_AttentionRotationBlock — full on-device pipeline.

Sharding (8 cores): core c handles batch b=c//4 with group rank r=c%4.
  - rmsnorm1 over the full batch is computed redundantly per core (cheap,
    avoids any gather), qkv is head-parallel: core computes q,k,v for its
    4 heads over all 2048 tokens of its batch.
  - causal attention for those 4 heads is fully local and perfectly
    balanced across cores; softmax runs in scores-transposed layout
    ([keys, queries]) with the denominator folded into the P@V matmul via
    a ones-column appended to V.
  - o-projection partials (own 256 hidden dims) are AllReduce-added (bf16)
    within each 4-core batch group; the tail (residual + rmsnorm2 + three
    rotation-GEMM/silu passes) is token-parallel on the core's own 512
    tokens.
All matmuls run in bf16 (fp32 PSUM accumulate).  Falls back to an exact
fp32 numpy path if the device path fails.
"""

import sys

import numpy as np

B, T, D, H, HD, NPASS = 2, 2048, 1024, 16, 64, 3
NCORES, R = 8, 4
TPC = T // R           # 512 own tokens per core
NKT = D // 128         # 8 feature tiles
NKB = T // 128         # 16 key blocks
NQC = 4                # query chunks of 512
QCW = T // NQC
HPC = H // R           # 4 heads per core
EPS = float(np.finfo(np.float32).eps)
GROUPS = [[0, 1, 2, 3], [4, 5, 6, 7]]
DEBUG = False


# ----------------------------------------------------------------------
# host reference pieces (fallback + input prep)
# ----------------------------------------------------------------------

def _rmsnorm(x, w):
    ms = np.mean(x * x, axis=-1, keepdims=True)
    return x * (1.0 / np.sqrt(ms + EPS)) * w


def _giv_mats(angles, pi, pj, gate):
    """Dense [D,D] G_p with gate folded in; rotated = r @ G_p."""
    mats = []
    for p in range(NPASS):
        G = np.eye(D, dtype=np.float64)
        ca = np.cos(angles[p].astype(np.float64))
        sa = np.sin(angles[p].astype(np.float64))
        ii = pi[p].astype(np.int64)
        jj = pj[p].astype(np.int64)
        G[ii, ii] = ca
        G[jj, ii] = -sa
        G[ii, jj] = sa
        G[jj, jj] = ca
        G = G * gate[p].astype(np.float64)[None, :]
        mats.append(G.astype(np.float32))
    return mats


def _host_full(x, scale_gamma, scale_beta, qkv_w, o_w, norm1_w, norm2_w,
               angles, gate, bias, pi, pj):
    xf = np.asarray(x, np.float32)
    h = _rmsnorm(xf, norm1_w) * scale_gamma + scale_beta
    qkv = (h.reshape(B * T, D) @ qkv_w.T).reshape(B, T, 3, H, HD)
    q = np.moveaxis(qkv[:, :, 0], 1, 2)
    k = np.moveaxis(qkv[:, :, 1], 1, 2)
    v = np.moveaxis(qkv[:, :, 2], 1, 2)
    scale = 1.0 / np.sqrt(HD)
    causal = np.tril(np.ones((T, T), bool))
    out = np.empty((B, H, T, HD), np.float32)
    for b in range(B):
        for hh in range(H):
            s = (q[b, hh] @ k[b, hh].T) * scale
            s = np.where(causal, s, -np.inf).astype(np.float32)
            s -= s.max(axis=-1, keepdims=True)
            e = np.exp(s)
            out[b, hh] = (e / e.sum(axis=-1, keepdims=True)) @ v[b, hh]
    ao = np.swapaxes(out, 1, 2).reshape(B, T, D)
    x2 = xf + ao @ o_w.T
    h2 = _rmsnorm(x2, norm2_w) * scale_gamma + scale_beta
    r = h2.reshape(B * T, D)
    for p, G in enumerate(_giv_mats(angles, pi, pj, gate)):
        r = r @ G + bias[p][None, :]
        r = r * (1.0 / (1.0 + np.exp(-r)))
    return (x2 + (r.reshape(B, T, D) - h2)).astype(np.float32)


# ----------------------------------------------------------------------
# walrus workaround: split multi-wait instructions (this container's
# walrus accepts only one embedded sync wait per instruction)
# ----------------------------------------------------------------------

def _split_multi_waits(nc, mybir, max_waits=1):
    n = 0
    for f in nc.m.functions:
        for bb in f.blocks:
            out = []
            for inst in bb.instructions:
                si = inst.sync_info
                if si is not None and si.on_wait and len(si.on_wait) > max_waits:
                    waits = list(si.on_wait)
                    for kk, w in enumerate(waits[:-max_waits]):
                        ev = mybir.InstEventSemaphore(
                            name=f"{inst.name}_xw{kk}",
                            engine=inst.engine,
                            ins=[],
                            outs=[],
                            sync_info=mybir.SyncInfo(on_wait=[w], on_update=[]),
                        )
                        out.append(ev)
                        n += 1
                    inst.sync_info = mybir.SyncInfo(
                        on_wait=waits[-max_waits:], on_update=list(si.on_update)
                    )
                out.append(inst)
            bb.instructions = out
    return n


# ----------------------------------------------------------------------
# device kernel
# ----------------------------------------------------------------------

def _build_nc():
    import concourse.bass as bass
    import concourse.mybir as mybir
    import concourse.tile as tile

    f32 = mybir.dt.float32
    bf16 = mybir.dt.bfloat16
    AF = mybir.ActivationFunctionType
    Alu = mybir.AluOpType

    nc = bass.Bass(num_devices=NCORES)

    xtb = nc.dram_tensor("xtb", [NKT, 128, T], bf16, kind="ExternalInput")
    xto = nc.dram_tensor("xto", [NKT, 128, TPC], f32, kind="ExternalInput")
    wqkv = nc.dram_tensor("wqkv", [NKT, 128, 768], bf16, kind="ExternalInput")
    wow = nc.dram_tensor("wow", [2, 128, D], bf16, kind="ExternalInput")
    gmat = nc.dram_tensor("gmat", [NPASS, NKT, 128, D], bf16,
                          kind="ExternalInput")
    geff1 = nc.dram_tensor("geff1", [128, NKT], f32, kind="ExternalInput")
    geff2 = nc.dram_tensor("geff2", [128, NKT], f32, kind="ExternalInput")
    betav = nc.dram_tensor("betav", [128, NKT], f32, kind="ExternalInput")
    biasr = nc.dram_tensor("biasr", [128, NKT, NPASS], f32,
                           kind="ExternalInput")
    trim = nc.dram_tensor("trim", [128, 2, 128], bf16, kind="ExternalInput")
    cqk = nc.dram_tensor("cqk", [128, 4], f32, kind="ExternalInput")
    cvb = nc.dram_tensor("cvb", [128, 256], bf16, kind="ExternalInput")
    rofs = nc.dram_tensor("rofs", [1, 2], mybir.dt.uint32,
                          kind="ExternalInput")
    yt = nc.dram_tensor("yt", [NKT, 128, TPC], f32, kind="ExternalOutput")
    if DEBUG:
        dbg_qk = nc.dram_tensor("dbg_qk", [128, 4, T], bf16,
                                kind="ExternalOutput")
        dbg_v = nc.dram_tensor("dbg_v", [128, NKB, HPC, HD + 1], bf16,
                               kind="ExternalOutput")
        dbg_at = nc.dram_tensor("dbg_at", [128, 2, T], bf16,
                                kind="ExternalOutput")
        dbg_rs = nc.dram_tensor("dbg_rs", [128, NKT, T], bf16,
                                kind="ExternalOutput")

    with tile.TileContext(nc) as tc:
        with (
            tc.tile_pool(name="big", bufs=1) as big,
            tc.tile_pool(name="med", bufs=2) as med,
            tc.tile_pool(name="small", bufs=1) as small,
            tc.tile_pool(name="ps", bufs=2, space="PSUM") as ps,
            tc.tile_pool(name="pso", bufs=2, space="PSUM") as pso,
            tc.tile_pool(name="dram", bufs=1, space="DRAM") as dram,
        ):
            # ---------------- constants / small loads ----------------
            ones_b = small.tile([128, 1], bf16, tag="ones_b")
            nc.vector.memset(ones_b[:, :], 1.0)
            # ones rows for K=1 partition-broadcast matmuls
            onr_b = small.tile([1, 128], bf16, tag="onr_b")
            nc.vector.memset(onr_b[:, :], 1.0)
            onr_f = small.tile([1, 128], f32, tag="onr_f")
            nc.vector.memset(onr_f[:, :], 1.0)
            eps_t = small.tile([1, 1], f32, tag="eps_t")
            nc.vector.memset(eps_t[:, :], EPS)
            g2_t = small.tile([128, NKT], f32, tag="g2_t")
            nc.sync.dma_start(out=g2_t[:, :], in_=geff2[:, :])
            be_t = small.tile([128, NKT], f32, tag="be_t")
            nc.sync.dma_start(out=be_t[:, :], in_=betav[:, :])
            br_t = small.tile([128, NKT, NPASS], f32, tag="br_t")
            nc.sync.dma_start(out=br_t[:, :, :], in_=biasr[:, :, :])
            tri_t = small.tile([128, 2, 128], bf16, tag="tri_t")
            nc.sync.dma_start(out=tri_t[:, :, :], in_=trim[:, :, :])

            # ---------------- phase A: load x^T; rmsnorm1 stats --------------
            # geff1 is folded into wqkv on the host and rstd commutes through
            # the qkv matmul as a per-token scalar, so qkv runs on raw x and
            # the norm statistics overlap it; rstd/beta are applied at PSUM
            # evacuation time.
            xt = big.tile([128, NKT, T], bf16, tag="xt")
            wq_t = big.tile([128, NKT, 768], bf16, tag="wq", bufs=2)
            nc.sync.dma_start(out=wq_t[:, :, :],
                              in_=wqkv[:, :, :].rearrange("k p t -> p k t"))
            cqk_t = small.tile([128, 4], f32, tag="cqk_t")
            nc.sync.dma_start(out=cqk_t[:, :], in_=cqk[:, :])
            cvb_t = small.tile([128, 256], bf16, tag="cvb_t")
            nc.sync.dma_start(out=cvb_t[:, :], in_=cvb[:, :])

            # per-chunk loads + norm stats are emitted inside the chunk loop
            # below so chunk 0's collective path is not queued behind the
            # other chunks' stats.  rstd = exp(-0.5 * ln(ssq/D + eps))
            # (one ACT table set; no sqrt table, no DVE reciprocal)
            lnms = small.tile([1, T], f32, tag="lnms")
            rstd = small.tile([1, T], bf16, tag="rstd")
            rstdB = small.tile([128, T], bf16, tag="rstdB")
            rtp_b = small.tile([128, NKB], f32, tag="rtp_b")

            # ---------------- phases B+C+D interleaved per query chunk -------
            # qkv for chunk qc -> attention chunk qc -> o-proj partial ->
            # chunked AllReduce (overlaps later chunks)
            wo_t = big.tile([128, 2, D], bf16, tag="wo")
            nc.sync.dma_start(out=wo_t[:, :, :],
                              in_=wow[:, :, :].rearrange("k p t -> p k t"))
            qkT = big.tile([128, 4, T], bf16, tag="qkT")
            v_sb = big.tile([128, NKB, HPC, HD + 1], bf16, tag="v_sb")
            nc.vector.memset(v_sb[:, :, :, HD:HD + 1], 1.0)
            attnT = big.tile([128, 2, T], bf16, tag="attnT")
            arin = [dram.tile([NKT, 128, QCW], bf16, name=f"arin{q}")
                    for q in range(NQC)]
            arout = [dram.tile([NKT, 128, QCW], bf16, name=f"arout{q}")
                    for q in range(NQC)]
            for qc in range(NQC):
                csl = slice(qc * QCW, (qc + 1) * QCW)
                # --- x loads + norm1 stats for this chunk ---
                for k in range(NKT):
                    nc.sync.dma_start(out=xt[:, k, csl], in_=xtb[k, :, csl])
                spool = pso if qc == 0 else ps
                stag = "out" if qc == 0 else "mm"
                ssq = spool.tile([1, QCW], f32, tag=stag, name=f"ssq{qc}")
                for k in range(NKT):
                    sq = med.tile([128, QCW], bf16, tag="sq")
                    nc.vector.tensor_mul(out=sq[:, :], in0=xt[:, k, csl],
                                         in1=xt[:, k, csl])
                    nc.tensor.matmul(ssq[:, :], ones_b[:, :], sq[:, :],
                                     start=(k == 0), stop=(k == NKT - 1))
                nc.scalar.activation(out=lnms[:, csl], in_=ssq[:, :],
                                     func=AF.Ln, scale=1.0 / D,
                                     bias=eps_t[:, :])
                nc.scalar.activation(out=rstd[:, csl], in_=lnms[:, csl],
                                     func=AF.Exp, scale=-0.5)
                bp = spool.tile([128, 2, QCW], f32, tag=stag,
                                name=f"bp{qc}")
                nc.tensor.matmul(bp[:, 0, :], onr_b[:, :], rstd[:, csl],
                                 start=True, stop=True)
                nc.vector.tensor_copy(out=rstdB[:, csl], in_=bp[:, 0, :])
                # rstd transposed to [token-partition, block] for v evac
                for i, tt in enumerate(range(4 * qc, 4 * qc + 4)):
                    nc.tensor.matmul(bp[:, 1, i:i + 1],
                                     rstd[:, tt * 128:(tt + 1) * 128],
                                     onr_b[:, 0:1],
                                     start=(i == 0), stop=(i == 3))
                nc.vector.tensor_copy(out=rtp_b[:, 4 * qc:4 * qc + 4],
                                      in_=bp[:, 1, 0:4])
                # --- qkv for this token chunk (m order 0,2,1,3 so head-pair
                # 0's q AND k land in the first slot wave) ---
                for m in (0, 2, 1, 3):  # 0,1: q head-pairs; 2,3: k pairs
                    qp = ps.tile([128, 2, QCW], f32, tag="mm",
                                 name=f"qk{qc}{m}")
                    for k in range(NKT):
                        nc.tensor.matmul(qp[:, 0, :],
                                         wq_t[:, k, m * 128:(m + 1) * 128],
                                         xt[:, k, csl],
                                         start=(k == 0), stop=(k == NKT - 1))
                    # qkT = rstd * raw + cqk[m]
                    nc.vector.tensor_mul(out=qkT[:, m, csl], in0=qp[:, 0, :],
                                         in1=rstdB[:, csl])
                    nc.vector.tensor_scalar(
                        out=qkT[:, m, csl], in0=qkT[:, m, csl],
                        scalar1=cqk_t[:, m:m + 1], scalar2=None, op0=Alu.add)
                # v blocks descending: the descending-j chain uses v[jmax]
                # first
                for tt in range(4 * qc + 3, 4 * qc - 1, -1):
                    vp = ps.tile([128, 2, QCW], f32, tag="mm", name=f"v{tt}")
                    for k in range(NKT):
                        nc.tensor.matmul(vp[:, 0, 0:256],
                                         xt[:, k, tt * 128:(tt + 1) * 128],
                                         wq_t[:, k, 512:768],
                                         start=(k == 0), stop=(k == NKT - 1))
                    nc.vector.tensor_scalar(
                        out=v_sb[:, tt, :, 0:HD],
                        in0=vp[:, 0, 0:256].rearrange("p (h d) -> p h d",
                                                      h=HPC),
                        scalar1=rtp_b[:, tt:tt + 1], scalar2=None,
                        op0=Alu.mult)
                    nc.vector.tensor_add(
                        out=v_sb[:, tt, :, 0:HD], in0=v_sb[:, tt, :, 0:HD],
                        in1=cvb_t[:, :].rearrange("p (h d) -> p h d", h=HPC))
                # --- attention for this chunk (both head-pair chains first,
                # then all divisions, so hp1's chain overlaps hp0's) ---
                outps = []
                jmax = 4 * qc + 3
                for hp in range(2):
                    outp = pso.tile([65, 2, QCW], f32, tag="out",
                                    name=f"o{hp}{qc}")
                    outps.append(outp)
                    for j in range(jmax, -1, -1):   # descending j
                        cs = max(0, (j - 4 * qc) * 128)
                        scp = ps.tile([128, 2, QCW], f32, tag="mm",
                                      name=f"s{hp}{qc}{j}")
                        for h01 in range(2):
                            pr = slice(64 * h01, 64 * h01 + 64)
                            nc.tensor.matmul(
                                scp[:, h01, cs:QCW],
                                qkT[pr, 2 + hp, j * 128:(j + 1) * 128],
                                qkT[pr, hp, qc * QCW + cs:(qc + 1) * QCW],
                                start=True, stop=True)
                        pt = med.tile([128, 2, QCW], bf16, tag="pt", bufs=3)
                        nc.scalar.activation(out=pt[:, :, cs:QCW],
                                             in_=scp[:, :, cs:QCW],
                                             func=AF.Exp, scale=0.125)
                        if j >= 4 * qc:
                            nc.vector.tensor_mul(out=pt[:, :, cs:cs + 128],
                                                 in0=pt[:, :, cs:cs + 128],
                                                 in1=tri_t[:, :, :])
                        for h01 in range(2):
                            h = 2 * hp + h01
                            nc.tensor.matmul(
                                outp[:, h01, cs:QCW],
                                v_sb[:, j, h, :],
                                pt[:, h01, cs:QCW],
                                start=(j == jmax), stop=(j == 0))
                # divisions, steps interleaved across the 4 heads
                dens, dbps, recs = [], [], []
                for hp in range(2):
                    for h01 in range(2):
                        den = med.tile([1, QCW], bf16, tag="den", bufs=4,
                                       name=f"dn{hp}{qc}{h01}")
                        # ACT is idle right after the exps; copying the
                        # denominator row there lets the division chain
                        # pipeline across three engines
                        nc.scalar.activation(out=den[:, :],
                                             in_=outps[hp][64:65, h01, :],
                                             func=AF.Copy)
                        dens.append(den)
                for hp in range(2):
                    for h01 in range(2):
                        dbp = ps.tile([128, 2, QCW], f32, tag="mm",
                                      name=f"db{hp}{qc}{h01}")
                        nc.tensor.matmul(dbp[0:64, 0, :], onr_b[:, 0:64],
                                         dens[2 * hp + h01][:, :],
                                         start=True, stop=True)
                        dbps.append(dbp)
                for hp in range(2):
                    for h01 in range(2):
                        rec = med.tile([64, QCW], f32, tag="rec", bufs=4,
                                       name=f"rc{hp}{qc}{h01}")
                        nc.vector.reciprocal(out=rec[:, :],
                                             in_=dbps[2 * hp + h01][0:64, 0, :])
                        recs.append(rec)
                for hp in range(2):
                    for h01 in range(2):
                        h = 2 * hp + h01
                        pr = slice(64 * (h % 2), 64 * (h % 2) + 64)
                        nc.vector.tensor_mul(
                            out=attnT[pr, h // 2, qc * QCW:(qc + 1) * QCW],
                            in0=outps[hp][0:64, h01, :],
                            in1=recs[2 * hp + h01][:, :])
                # o-proj partial for this query chunk, then its AllReduce
                dst = med.tile([128, NKT, QCW], bf16, tag="dst")
                for m in range(NKT):
                    dp = ps.tile([128, 2, QCW], f32, tag="mm",
                                 name=f"d{qc}{m}")
                    for k in range(2):
                        nc.tensor.matmul(dp[:, 0, :],
                                         wo_t[:, k, m * 128:(m + 1) * 128],
                                         attnT[:, k, qc * QCW:(qc + 1) * QCW],
                                         start=(k == 0), stop=(k == 1))
                    if m % 2 == 0 and qc < NQC - 1:
                        nc.vector.tensor_copy(out=dst[:, m, :],
                                              in_=dp[:, 0, :])
                    else:
                        # last chunk: all evacuations on ACT to keep DVE
                        # free for the division chain
                        nc.scalar.activation(out=dst[:, m, :],
                                             in_=dp[:, 0, :], func=AF.Copy)
                    nc.sync.dma_start(out=arin[qc][m, :, :],
                                      in_=dst[:, m, :])
                nc.gpsimd.collective_compute(
                    "AllReduce", Alu.add,
                    replica_groups=GROUPS,
                    ins=[arin[qc].opt()], outs=[arout[qc].opt()],
                )

            # ---------------- phase E: tail on own 512 tokens ----------------
            # the AllReduce result is identical across the group; each core
            # reads it all (reusing xt's slot) and selects its own 512-token
            # slice with a runtime register offset (rofs, host-supplied).
            rsb = big.tile([128, NKT, T], bf16, tag="xt", name="rsb")
            for q in range(NQC):
                # gpsimd (SWDGE) so these collective-gated reads never stall
                # the sync queue that carries the arin bounce writes; per-k
                # so the x2 adds pipeline with the last chunk's read
                for k in range(NKT):
                    nc.gpsimd.dma_start(
                        out=rsb[:, k, q * QCW:(q + 1) * QCW],
                        in_=arout[q][k, :, :])
            if DEBUG:
                nc.sync.dma_start(out=dbg_qk[:, :, :], in_=qkT[:, :, :])
                nc.sync.dma_start(out=dbg_v[:, :, :, :], in_=v_sb[:, :, :, :])
                nc.sync.dma_start(out=dbg_at[:, :, :], in_=attnT[:, :, :])
                nc.sync.dma_start(out=dbg_rs[:, :, :], in_=rsb[:, :, :])
            rreg = nc.vector.alloc_register("rofs_reg")
            nc.vector.reg_load(rreg, rofs[0:1, 0:1])
            roff = nc.vector.snap(rreg, donate=True, min_val=0,
                                  max_val=T - TPC)
            x2 = big.tile([128, NKT, TPC], f32, tag="x2")
            x2b = big.tile([128, NKT, TPC], bf16, tag="h2", name="x2b")
            # rmsnorm2 stats (same ln/exp trick); the x2b half-0 casts are
            # emitted first so rotation pass 0 (which needs only half 0)
            # starts asap, then stats follow
            lnms2 = small.tile([1, TPC], f32, tag="lnms2")
            rstd2 = small.tile([1, TPC], bf16, tag="rstd2")
            rstd2B = small.tile([128, TPC], bf16, tag="rstd2B")
            bp2 = pso.tile([128, 2, QCW], f32, tag="out", name="bp2")
            h0 = slice(0, TPC // 2)
            h1 = slice(TPC // 2, TPC)
            for k in range(NKT):
                xo_k = med.tile([128, TPC], f32, tag="xo")
                nc.sync.dma_start(out=xo_k[:, :], in_=xto[k, :, :])
                nc.vector.tensor_add(out=x2[:, k, :], in0=xo_k[:, :],
                                     in1=rsb[:, k, bass.ds(roff, TPC)])
                nc.vector.tensor_copy(out=x2b[:, k, h0], in_=x2[:, k, h0])
            ssq2 = pso.tile([1, TPC], f32, tag="out", name="ssq2")
            for k in range(NKT):
                nc.vector.tensor_copy(out=x2b[:, k, h1], in_=x2[:, k, h1])
                sq2 = med.tile([128, TPC], bf16, tag="sq2")
                nc.scalar.activation(out=sq2[:, :], in_=x2[:, k, :],
                                     func=AF.Square)
                nc.tensor.matmul(ssq2[:, :], ones_b[:, :], sq2[:, :],
                                 start=(k == 0), stop=(k == NKT - 1))
            nc.scalar.activation(out=lnms2[:, :], in_=ssq2[:, :],
                                 func=AF.Ln, scale=1.0 / D, bias=eps_t[:, :])
            nc.scalar.activation(out=rstd2[:, :], in_=lnms2[:, :],
                                 func=AF.Exp, scale=-0.5)
            nc.tensor.matmul(bp2[:, 0, :], onr_b[:, :], rstd2[:, :],
                             start=True, stop=True)
            nc.vector.tensor_copy(out=rstd2B[:, :], in_=bp2[:, 0, :])
            # geff2 is folded into G_0 on the host (plus a beta@G_0 bias
            # shift) and rstd2 commutes through the pass-0 matmul as a
            # per-token scalar: pass 0 runs on raw bf16 x2 (cast above, no
            # wait on the norm2 stats); rstd2 is applied at PSUM evacuation.

            # rotation passes (gate folded into G; bias separate).  G for
            # pass 0/2 lands in the freed wq slots, pass 1 in the freed qkT
            # slot; all three loads are emitted early so they prefetch
            # behind the AllReduce chain.
            gh01 = big.tile([128, 4, D], bf16, tag="wq", bufs=2, name="gh01")
            nc.sync.dma_start(out=gh01[:, :, :],
                              in_=gmat[0, 0:4, :, :].rearrange("k p t -> p k t"))
            gh00 = big.tile([128, 4, D], bf16, tag="wq", bufs=2, name="gh00")
            nc.sync.dma_start(out=gh00[:, :, :],
                              in_=gmat[0, 4:8, :, :].rearrange("k p t -> p k t"))
            gh1 = big.tile([128, NKT, D], bf16, tag="qkT", name="gh1")
            nc.sync.dma_start(out=gh1[:, :, :],
                              in_=gmat[1, :, :, :].rearrange("k p t -> p k t"))
            gslices = {
                0: [gh01[:, :, :], gh00[:, :, :]],
                1: [gh1[:, 0:4, :], gh1[:, 4:8, :]],
            }
            rb0 = big.tile([128, NKT, TPC], bf16, tag="rb0")
            rb1 = big.tile([128, NKT, TPC], bf16, tag="rb1")
            cur = x2b
            for p3 in range(NPASS):
                last = p3 == NPASS - 1
                nxt = None if last else (rb0 if p3 == 0 else rb1)
                if p3 < 2:
                    gh = gslices[p3]
                else:
                    gh = []
                    for half in range(2):
                        gt_ = big.tile([128, 4, D], bf16, tag="wq", bufs=2,
                                       name=f"gh2{half}")
                        nc.sync.dma_start(
                            out=gt_[:, :, :],
                            in_=gmat[2, 4 * half:4 * half + 4, :, :]
                            .rearrange("k p t -> p k t"))
                        gh.append(gt_[:, :, :])
                # token-halved so consecutive passes pipeline: pass p+1 on
                # half 0 starts once pass p finished half 0
                for th in range(2):
                    tsl = slice(th * (TPC // 2), (th + 1) * (TPC // 2))
                    for m in range(NKT):
                        rp = ps.tile([128, 2, QCW], f32, tag="mm",
                                     name=f"r{p3}{m}{th}")
                        for k in range(NKT):
                            nc.tensor.matmul(
                                rp[:, 0, 0:TPC // 2],
                                gh[k // 4][:, k % 4,
                                           m * 128:(m + 1) * 128],
                                cur[:, k, tsl],
                                start=(k == 0), stop=(k == NKT - 1))
                        if p3 == 0:
                            # apply the commuted rstd2 before the silu
                            rn = med.tile([128, TPC // 2], bf16, tag="rn")
                            nc.vector.tensor_mul(out=rn[:, :],
                                                 in0=rp[:, 0, 0:TPC // 2],
                                                 in1=rstd2B[:, tsl])
                            nc.scalar.activation(
                                out=nxt[:, m, tsl], in_=rn[:, :],
                                func=AF.Silu, bias=br_t[:, m, p3:p3 + 1])
                        elif not last:
                            nc.scalar.activation(
                                out=nxt[:, m, tsl], in_=rp[:, 0, 0:TPC // 2],
                                func=AF.Silu, bias=br_t[:, m, p3:p3 + 1])
                        else:
                            # final: y_m = (x2_m - h2_m) + silu(rp); the
                            # subtraction was folded into x2 during pass 1
                            rl = med.tile([128, TPC // 2], f32, tag="rl")
                            nc.scalar.activation(
                                out=rl[:, :], in_=rp[:, 0, 0:TPC // 2],
                                func=AF.Silu, bias=br_t[:, m, p3:p3 + 1])
                            nc.vector.tensor_add(out=x2[:, m, tsl],
                                                 in0=x2[:, m, tsl],
                                                 in1=rl[:, :])
                    if last:
                        nc.sync.dma_start(
                            out=yt[:, :, tsl].rearrange("k p t -> p k t"),
                            in_=x2[:, :, tsl])
                if p3 == 0:
                    # h2 and the (x2 - h2) fold, overlapped with pass 1
                    for k in range(NKT):
                        h2k = med.tile([128, TPC], bf16, tag="h2k", bufs=2,
                                       name=f"h2k{k}")
                        nc.vector.tensor_mul(out=h2k[:, :],
                                             in0=x2b[:, k, :],
                                             in1=rstd2B[:, :])
                        nc.vector.tensor_scalar(
                            out=h2k[:, :], in0=h2k[:, :],
                            scalar1=g2_t[:, k:k + 1],
                            scalar2=be_t[:, k:k + 1],
                            op0=Alu.mult, op1=Alu.add)
                        nc.vector.tensor_sub(out=x2[:, k, :],
                                             in0=x2[:, k, :],
                                             in1=h2k[:, :])
                cur = nxt
    return nc


def _prep_inputs(x, scale_gamma, scale_beta, qkv_w, o_w, norm1_w, norm2_w,
                 angles, gate, bias, pi, pj):
    import ml_dtypes
    bf = ml_dtypes.bfloat16
    x = np.asarray(x, np.float32)
    gmats = _giv_mats(np.asarray(angles), np.asarray(pi), np.asarray(pj),
                      np.asarray(gate))
    g1 = (np.asarray(norm1_w) * np.asarray(scale_gamma)).astype(np.float32)
    g2 = (np.asarray(norm2_w) * np.asarray(scale_gamma)).astype(np.float32)
    beta = np.asarray(scale_beta, np.float32)
    # fold geff2 into G_0 (rotation pass 0 runs on x2n = x2*rstd2); the
    # beta part of h2 projects into the pass-0 bias
    bias = np.asarray(bias, np.float32).copy()
    bias[0] = bias[0] + beta @ gmats[0]
    gmats[0] = g2[:, None] * gmats[0]
    gm = np.stack([g.reshape(NKT, 128, D) for g in gmats]).astype(bf)
    g1_t = np.ascontiguousarray(g1.reshape(NKT, 128).T)
    g2_t = np.ascontiguousarray(g2.reshape(NKT, 128).T)
    be_t = np.ascontiguousarray(beta.reshape(NKT, 128).T)
    br = bias.reshape(NPASS, NKT, 128)
    br_t = np.ascontiguousarray(np.transpose(br, (2, 1, 0)))
    tri = np.tril(np.ones((128, 128), np.float32)).T  # [kl, ql] kl<=ql
    tri2 = np.ascontiguousarray(
        np.broadcast_to(tri[:, None, :], (128, 2, 128))).astype(bf)
    qkv_wT = np.asarray(qkv_w, np.float32).T  # [D, 3D]
    o_wT = np.asarray(o_w, np.float32).T      # [D(d), D(j)]

    in_maps = []
    for c in range(NCORES):
        b, r = divmod(c, R)
        xT = np.ascontiguousarray(x[b].T)          # [D, T]
        hsel = np.r_[256 * r:256 * r + 256]
        wsel = np.concatenate([qkv_wT[:, hsel], qkv_wT[:, 1024 + hsel],
                               qkv_wT[:, 2048 + hsel]], axis=1)  # [D, 768]
        # fold geff1 into the weight rows; beta projects to a const vector
        wfold = wsel * g1[:, None]
        cvec = beta @ wsel                          # [768]
        cqk_m = np.ascontiguousarray(
            cvec[:512].reshape(4, 128).T).astype(np.float32)   # [128, 4]
        cvb_m = np.ascontiguousarray(
            np.broadcast_to(cvec[512:768], (128, 256))).astype(bf)
        m = dict(
            xtb=xT.reshape(NKT, 128, T).astype(bf),
            xto=np.ascontiguousarray(
                xT[:, r * TPC:(r + 1) * TPC]).reshape(NKT, 128, TPC),
            wqkv=np.ascontiguousarray(wfold).reshape(NKT, 128, 768).astype(bf),
            wow=np.ascontiguousarray(
                o_wT[256 * r:256 * r + 256]).reshape(2, 128, D).astype(bf),
            gmat=gm,
            geff1=g1_t, geff2=g2_t, betav=be_t, biasr=br_t,
            trim=tri2,
            cqk=cqk_m, cvb=cvb_m,
            rofs=np.array([[r * TPC, r * TPC + TPC // 2]], np.uint32),
        )
        in_maps.append(m)
    return in_maps


_NC_CACHE = [None]


def _device_run(in_maps):
    from concourse import bass_utils
    import concourse.mybir as mybir

    if _NC_CACHE[0] is None:
        nc = _build_nc()
        _split_multi_waits(nc, mybir)
        _NC_CACHE[0] = nc
    nc = _NC_CACHE[0]
    res = bass_utils.run_bass_kernel_spmd(nc, in_maps,
                                          core_ids=list(range(NCORES)))
    y = np.empty((B, T, D), np.float32)
    for c in range(NCORES):
        b, r = divmod(c, R)
        yT = res.results[c]["yt"].reshape(D, TPC)
        y[b, r * TPC:(r + 1) * TPC] = yT.T
    return y


def kernel(x, scale_gamma, scale_beta, qkv_w, o_w, norm1_w, norm2_w,
           angles, gate, bias, pi, pj):
    args = dict(x=x, scale_gamma=scale_gamma, scale_beta=scale_beta,
                qkv_w=qkv_w, o_w=o_w, norm1_w=norm1_w, norm2_w=norm2_w,
                angles=angles, gate=gate, bias=bias, pi=pi, pj=pj)
    try:
        in_maps = _prep_inputs(**args)
        return _device_run(in_maps)
    except Exception as e:
        print(f"device path failed ({type(e).__name__}: {e}); "
              "using host fallback", file=sys.stderr)
        import traceback
        traceback.print_exc()
        return _host_full(**args)


# revision 81
# speedup vs baseline: 1.0043x; 1.0027x over previous
"""Trainium2 kernel for nn_AttentionRotationBlock — full on-device pipeline.

Sharding (8 cores): core c handles batch b=c//4 with group rank r=c%4.
  - rmsnorm1 over the full batch is computed redundantly per core (cheap,
    avoids any gather), qkv is head-parallel: core computes q,k,v for its
    4 heads over all 2048 tokens of its batch.
  - causal attention for those 4 heads is fully local and perfectly
    balanced across cores; softmax runs in scores-transposed layout
    ([keys, queries]) with the denominator folded into the P@V matmul via
    a ones-column appended to V.
  - o-projection partials (own 256 hidden dims) are AllReduce-added (bf16)
    within each 4-core batch group; the tail (residual + rmsnorm2 + three
    rotation-GEMM/silu passes) is token-parallel on the core's own 512
    tokens.
All matmuls run in bf16 (fp32 PSUM accumulate).  Falls back to an exact
fp32 numpy path if the device path fails.
"""

import sys

import numpy as np

B, T, D, H, HD, NPASS = 2, 2048, 1024, 16, 64, 3
NCORES, R = 8, 4
TPC = T // R           # 512 own tokens per core
NKT = D // 128         # 8 feature tiles
NKB = T // 128         # 16 key blocks
NQC = 4                # query chunks of 512
QCW = T // NQC
HPC = H // R           # 4 heads per core
EPS = float(np.finfo(np.float32).eps)
GROUPS = [[0, 1, 2, 3], [4, 5, 6, 7]]
DEBUG = False


# ----------------------------------------------------------------------
# host reference pieces (fallback + input prep)
# ----------------------------------------------------------------------

def _rmsnorm(x, w):
    ms = np.mean(x * x, axis=-1, keepdims=True)
    return x * (1.0 / np.sqrt(ms + EPS)) * w


def _giv_mats(angles, pi, pj, gate):
    """Dense [D,D] G_p with gate folded in; rotated = r @ G_p."""
    mats = []
    for p in range(NPASS):
        G = np.eye(D, dtype=np.float64)
        ca = np.cos(angles[p].astype(np.float64))
        sa = np.sin(angles[p].astype(np.float64))
        ii = pi[p].astype(np.int64)
        jj = pj[p].astype(np.int64)
        G[ii, ii] = ca
        G[jj, ii] = -sa
        G[ii, jj] = sa
        G[jj, jj] = ca
        G = G * gate[p].astype(np.float64)[None, :]
        mats.append(G.astype(np.float32))
    return mats


def _host_full(x, scale_gamma, scale_beta, qkv_w, o_w, norm1_w, norm2_w,
               angles, gate, bias, pi, pj):
    xf = np.asarray(x, np.float32)
    h = _rmsnorm(xf, norm1_w) * scale_gamma + scale_beta
    qkv = (h.reshape(B * T, D) @ qkv_w.T).reshape(B, T, 3, H, HD)
    q = np.moveaxis(qkv[:, :, 0], 1, 2)
    k = np.moveaxis(qkv[:, :, 1], 1, 2)
    v = np.moveaxis(qkv[:, :, 2], 1, 2)
    scale = 1.0 / np.sqrt(HD)
    causal = np.tril(np.ones((T, T), bool))
    out = np.empty((B, H, T, HD), np.float32)
    for b in range(B):
        for hh in range(H):
            s = (q[b, hh] @ k[b, hh].T) * scale
            s = np.where(causal, s, -np.inf).astype(np.float32)
            s -= s.max(axis=-1, keepdims=True)
            e = np.exp(s)
            out[b, hh] = (e / e.sum(axis=-1, keepdims=True)) @ v[b, hh]
    ao = np.swapaxes(out, 1, 2).reshape(B, T, D)
    x2 = xf + ao @ o_w.T
    h2 = _rmsnorm(x2, norm2_w) * scale_gamma + scale_beta
    r = h2.reshape(B * T, D)
    for p, G in enumerate(_giv_mats(angles, pi, pj, gate)):
        r = r @ G + bias[p][None, :]
        r = r * (1.0 / (1.0 + np.exp(-r)))
    return (x2 + (r.reshape(B, T, D) - h2)).astype(np.float32)


# ----------------------------------------------------------------------
# walrus workaround: split multi-wait instructions (this container's
# walrus accepts only one embedded sync wait per instruction)
# ----------------------------------------------------------------------

def _split_multi_waits(nc, mybir, max_waits=1):
    n = 0
    for f in nc.m.functions:
        for bb in f.blocks:
            out = []
            for inst in bb.instructions:
                si = inst.sync_info
                if si is not None and si.on_wait and len(si.on_wait) > max_waits:
                    waits = list(si.on_wait)
                    for kk, w in enumerate(waits[:-max_waits]):
                        ev = mybir.InstEventSemaphore(
                            name=f"{inst.name}_xw{kk}",
                            engine=inst.engine,
                            ins=[],
                            outs=[],
                            sync_info=mybir.SyncInfo(on_wait=[w], on_update=[]),
                        )
                        out.append(ev)
                        n += 1
                    inst.sync_info = mybir.SyncInfo(
                        on_wait=waits[-max_waits:], on_update=list(si.on_update)
                    )
                out.append(inst)
            bb.instructions = out
    return n


# ----------------------------------------------------------------------
# device kernel
# ----------------------------------------------------------------------

def _build_nc():
    import concourse.bass as bass
    import concourse.mybir as mybir
    import concourse.tile as tile

    f32 = mybir.dt.float32
    bf16 = mybir.dt.bfloat16
    AF = mybir.ActivationFunctionType
    Alu = mybir.AluOpType

    nc = bass.Bass(num_devices=NCORES)

    xtb = nc.dram_tensor("xtb", [NKT, 128, T], bf16, kind="ExternalInput")
    xto = nc.dram_tensor("xto", [NKT, 128, TPC], f32, kind="ExternalInput")
    wqkv = nc.dram_tensor("wqkv", [NKT, 128, 768], bf16, kind="ExternalInput")
    wow = nc.dram_tensor("wow", [2, 128, D], bf16, kind="ExternalInput")
    gmat = nc.dram_tensor("gmat", [NPASS, NKT, 128, D], bf16,
                          kind="ExternalInput")
    geff1 = nc.dram_tensor("geff1", [128, NKT], f32, kind="ExternalInput")
    geff2 = nc.dram_tensor("geff2", [128, NKT], f32, kind="ExternalInput")
    betav = nc.dram_tensor("betav", [128, NKT], f32, kind="ExternalInput")
    biasr = nc.dram_tensor("biasr", [128, NKT, NPASS], f32,
                           kind="ExternalInput")
    trim = nc.dram_tensor("trim", [128, 2, 128], bf16, kind="ExternalInput")
    cqk = nc.dram_tensor("cqk", [128, 4], f32, kind="ExternalInput")
    cvb = nc.dram_tensor("cvb", [128, 256], bf16, kind="ExternalInput")
    rofs = nc.dram_tensor("rofs", [1, 2], mybir.dt.uint32,
                          kind="ExternalInput")
    yt = nc.dram_tensor("yt", [NKT, 128, TPC], f32, kind="ExternalOutput")
    if DEBUG:
        dbg_qk = nc.dram_tensor("dbg_qk", [128, 4, T], bf16,
                                kind="ExternalOutput")
        dbg_v = nc.dram_tensor("dbg_v", [128, NKB, HPC, HD + 1], bf16,
                               kind="ExternalOutput")
        dbg_at = nc.dram_tensor("dbg_at", [128, 2, T], bf16,
                                kind="ExternalOutput")
        dbg_rs = nc.dram_tensor("dbg_rs", [128, NKT, T], bf16,
                                kind="ExternalOutput")

    with tile.TileContext(nc) as tc:
        with (
            tc.tile_pool(name="big", bufs=1) as big,
            tc.tile_pool(name="med", bufs=2) as med,
            tc.tile_pool(name="small", bufs=1) as small,
            tc.tile_pool(name="ps", bufs=2, space="PSUM") as ps,
            tc.tile_pool(name="pso", bufs=2, space="PSUM") as pso,
            tc.tile_pool(name="dram", bufs=1, space="DRAM") as dram,
        ):
            # ---------------- constants / small loads ----------------
            ones_b = small.tile([128, 1], bf16, tag="ones_b")
            nc.vector.memset(ones_b[:, :], 1.0)
            # ones rows for K=1 partition-broadcast matmuls
            onr_b = small.tile([1, 128], bf16, tag="onr_b")
            nc.vector.memset(onr_b[:, :], 1.0)
            onr_f = small.tile([1, 128], f32, tag="onr_f")
            nc.vector.memset(onr_f[:, :], 1.0)
            eps_t = small.tile([1, 1], f32, tag="eps_t")
            nc.vector.memset(eps_t[:, :], EPS)
            g2_t = small.tile([128, NKT], f32, tag="g2_t")
            nc.sync.dma_start(out=g2_t[:, :], in_=geff2[:, :])
            be_t = small.tile([128, NKT], f32, tag="be_t")
            nc.sync.dma_start(out=be_t[:, :], in_=betav[:, :])
            br_t = small.tile([128, NKT, NPASS], f32, tag="br_t")
            nc.sync.dma_start(out=br_t[:, :, :], in_=biasr[:, :, :])
            tri_t = small.tile([128, 2, 128], bf16, tag="tri_t")
            nc.sync.dma_start(out=tri_t[:, :, :], in_=trim[:, :, :])

            # ---------------- phase A: load x^T; rmsnorm1 stats --------------
            # geff1 is folded into wqkv on the host and rstd commutes through
            # the qkv matmul as a per-token scalar, so qkv runs on raw x and
            # the norm statistics overlap it; rstd/beta are applied at PSUM
            # evacuation time.
            xt = big.tile([128, NKT, T], bf16, tag="xt")
            wq_t = big.tile([128, NKT, 768], bf16, tag="wq", bufs=2)
            nc.sync.dma_start(out=wq_t[:, :, :],
                              in_=wqkv[:, :, :].rearrange("k p t -> p k t"))
            cqk_t = small.tile([128, 4], f32, tag="cqk_t")
            nc.sync.dma_start(out=cqk_t[:, :], in_=cqk[:, :])
            cvb_t = small.tile([128, 256], bf16, tag="cvb_t")
            nc.sync.dma_start(out=cvb_t[:, :], in_=cvb[:, :])

            # per-chunk loads + norm stats are emitted inside the chunk loop
            # below so chunk 0's collective path is not queued behind the
            # other chunks' stats.  rstd = exp(-0.5 * ln(ssq/D + eps))
            # (one ACT table set; no sqrt table, no DVE reciprocal)
            lnms = small.tile([1, T], f32, tag="lnms")
            rstd = small.tile([1, T], bf16, tag="rstd")
            rstdB = small.tile([128, T], bf16, tag="rstdB")
            rtp_b = small.tile([128, NKB], f32, tag="rtp_b")

            # ---------------- phases B+C+D interleaved per query chunk -------
            # qkv for chunk qc -> attention chunk qc -> o-proj partial ->
            # chunked AllReduce (overlaps later chunks)
            wo_t = big.tile([128, 2, D], bf16, tag="wo")
            nc.sync.dma_start(out=wo_t[:, :, :],
                              in_=wow[:, :, :].rearrange("k p t -> p k t"))
            qkT = big.tile([128, 4, T], bf16, tag="qkT")
            v_sb = big.tile([128, NKB, HPC, HD + 1], bf16, tag="v_sb")
            nc.vector.memset(v_sb[:, :, :, HD:HD + 1], 1.0)
            attnT = big.tile([128, 2, T], bf16, tag="attnT")
            arin = [dram.tile([NKT, 128, QCW], bf16, name=f"arin{q}")
                    for q in range(NQC)]
            arout = [dram.tile([NKT, 128, QCW], bf16, name=f"arout{q}")
                    for q in range(NQC)]
            for qc in range(NQC):
                csl = slice(qc * QCW, (qc + 1) * QCW)
                # --- x loads + norm1 stats for this chunk ---
                for k in range(NKT):
                    nc.sync.dma_start(out=xt[:, k, csl], in_=xtb[k, :, csl])
                spool = pso if qc == 0 else ps
                stag = "out" if qc == 0 else "mm"
                ssq = spool.tile([1, QCW], f32, tag=stag, name=f"ssq{qc}")
                for k in range(NKT):
                    sq = med.tile([128, QCW], bf16, tag="sq")
                    nc.vector.tensor_mul(out=sq[:, :], in0=xt[:, k, csl],
                                         in1=xt[:, k, csl])
                    nc.tensor.matmul(ssq[:, :], ones_b[:, :], sq[:, :],
                                     start=(k == 0), stop=(k == NKT - 1))
                nc.scalar.activation(out=lnms[:, csl], in_=ssq[:, :],
                                     func=AF.Ln, scale=1.0 / D,
                                     bias=eps_t[:, :])
                nc.scalar.activation(out=rstd[:, csl], in_=lnms[:, csl],
                                     func=AF.Exp, scale=-0.5)
                bp = spool.tile([128, 2, QCW], f32, tag=stag,
                                name=f"bp{qc}")
                nc.tensor.matmul(bp[:, 0, :], onr_b[:, :], rstd[:, csl],
                                 start=True, stop=True)
                nc.vector.tensor_copy(out=rstdB[:, csl], in_=bp[:, 0, :])
                # rstd transposed to [token-partition, block] for v evac
                for i, tt in enumerate(range(4 * qc, 4 * qc + 4)):
                    nc.tensor.matmul(bp[:, 1, i:i + 1],
                                     rstd[:, tt * 128:(tt + 1) * 128],
                                     onr_b[:, 0:1],
                                     start=(i == 0), stop=(i == 3))
                nc.vector.tensor_copy(out=rtp_b[:, 4 * qc:4 * qc + 4],
                                      in_=bp[:, 1, 0:4])
                # --- qkv for this token chunk (m order 0,2,1,3 so head-pair
                # 0's q AND k land in the first slot wave) ---
                for m in (0, 2, 1, 3):  # 0,1: q head-pairs; 2,3: k pairs
                    qp = ps.tile([128, 2, QCW], f32, tag="mm",
                                 name=f"qk{qc}{m}")
                    for k in range(NKT):
                        nc.tensor.matmul(qp[:, 0, :],
                                         wq_t[:, k, m * 128:(m + 1) * 128],
                                         xt[:, k, csl],
                                         start=(k == 0), stop=(k == NKT - 1))
                    # qkT = rstd * raw + cqk[m]
                    nc.vector.tensor_mul(out=qkT[:, m, csl], in0=qp[:, 0, :],
                                         in1=rstdB[:, csl])
                    nc.vector.tensor_scalar(
                        out=qkT[:, m, csl], in0=qkT[:, m, csl],
                        scalar1=cqk_t[:, m:m + 1], scalar2=None, op0=Alu.add)
                # v blocks descending: the descending-j chain uses v[jmax]
                # first
                for tt in range(4 * qc + 3, 4 * qc - 1, -1):
                    vp = ps.tile([128, 2, QCW], f32, tag="mm", name=f"v{tt}")
                    for k in range(NKT):
                        nc.tensor.matmul(vp[:, 0, 0:256],
                                         xt[:, k, tt * 128:(tt + 1) * 128],
                                         wq_t[:, k, 512:768],
                                         start=(k == 0), stop=(k == NKT - 1))
                    nc.vector.tensor_scalar(
                        out=v_sb[:, tt, :, 0:HD],
                        in0=vp[:, 0, 0:256].rearrange("p (h d) -> p h d",
                                                      h=HPC),
                        scalar1=rtp_b[:, tt:tt + 1], scalar2=None,
                        op0=Alu.mult)
                    nc.vector.tensor_add(
                        out=v_sb[:, tt, :, 0:HD], in0=v_sb[:, tt, :, 0:HD],
                        in1=cvb_t[:, :].rearrange("p (h d) -> p h d", h=HPC))
                # --- attention for this chunk (both head-pair chains first,
                # then all divisions, so hp1's chain overlaps hp0's) ---
                outps = []
                jmax = 4 * qc + 3
                for hp in range(2):
                    outp = pso.tile([65, 2, QCW], f32, tag="out",
                                    name=f"o{hp}{qc}")
                    outps.append(outp)
                    for j in range(jmax, -1, -1):   # descending j
                        cs = max(0, (j - 4 * qc) * 128)
                        scp = ps.tile([128, 2, QCW], f32, tag="mm",
                                      name=f"s{hp}{qc}{j}")
                        for h01 in range(2):
                            pr = slice(64 * h01, 64 * h01 + 64)
                            nc.tensor.matmul(
                                scp[:, h01, cs:QCW],
                                qkT[pr, 2 + hp, j * 128:(j + 1) * 128],
                                qkT[pr, hp, qc * QCW + cs:(qc + 1) * QCW],
                                start=True, stop=True)
                        pt = med.tile([128, 2, QCW], bf16, tag="pt", bufs=3)
                        nc.scalar.activation(out=pt[:, :, cs:QCW],
                                             in_=scp[:, :, cs:QCW],
                                             func=AF.Exp, scale=0.125)
                        if j >= 4 * qc:
                            nc.vector.tensor_mul(out=pt[:, :, cs:cs + 128],
                                                 in0=pt[:, :, cs:cs + 128],
                                                 in1=tri_t[:, :, :])
                        for h01 in range(2):
                            h = 2 * hp + h01
                            nc.tensor.matmul(
                                outp[:, h01, cs:QCW],
                                v_sb[:, j, h, :],
                                pt[:, h01, cs:QCW],
                                start=(j == jmax), stop=(j == 0))
                # divisions, steps interleaved across the 4 heads
                dens, dbps, recs = [], [], []
                for hp in range(2):
                    for h01 in range(2):
                        den = med.tile([1, QCW], bf16, tag="den", bufs=4,
                                       name=f"dn{hp}{qc}{h01}")
                        # ACT is idle right after the exps; copying the
                        # denominator row there lets the division chain
                        # pipeline across three engines
                        nc.scalar.activation(out=den[:, :],
                                             in_=outps[hp][64:65, h01, :],
                                             func=AF.Copy)
                        dens.append(den)
                for hp in range(2):
                    for h01 in range(2):
                        dbp = ps.tile([128, 2, QCW], f32, tag="mm",
                                      name=f"db{hp}{qc}{h01}")
                        nc.tensor.matmul(dbp[0:64, 0, :], onr_b[:, 0:64],
                                         dens[2 * hp + h01][:, :],
                                         start=True, stop=True)
                        dbps.append(dbp)
                for hp in range(2):
                    for h01 in range(2):
                        rec = med.tile([64, QCW], f32, tag="rec", bufs=4,
                                       name=f"rc{hp}{qc}{h01}")
                        nc.vector.reciprocal(out=rec[:, :],
                                             in_=dbps[2 * hp + h01][0:64, 0, :])
                        recs.append(rec)
                for hp in range(2):
                    for h01 in range(2):
                        h = 2 * hp + h01
                        pr = slice(64 * (h % 2), 64 * (h % 2) + 64)
                        nc.vector.tensor_mul(
                            out=attnT[pr, h // 2, qc * QCW:(qc + 1) * QCW],
                            in0=outps[hp][0:64, h01, :],
                            in1=recs[2 * hp + h01][:, :])
                # o-proj partial for this query chunk, then its AllReduce
                dst = med.tile([128, NKT, QCW], bf16, tag="dst")
                for m in range(NKT):
                    dp = ps.tile([128, 2, QCW], f32, tag="mm",
                                 name=f"d{qc}{m}")
                    for k in range(2):
                        nc.tensor.matmul(dp[:, 0, :],
                                         wo_t[:, k, m * 128:(m + 1) * 128],
                                         attnT[:, k, qc * QCW:(qc + 1) * QCW],
                                         start=(k == 0), stop=(k == 1))
                    if m % 2 == 0 and qc < NQC - 1:
                        nc.vector.tensor_copy(out=dst[:, m, :],
                                              in_=dp[:, 0, :])
                    else:
                        # last chunk: all evacuations on ACT to keep DVE
                        # free for the division chain
                        nc.scalar.activation(out=dst[:, m, :],
                                             in_=dp[:, 0, :], func=AF.Copy)
                    nc.sync.dma_start(out=arin[qc][m, :, :],
                                      in_=dst[:, m, :])
                nc.gpsimd.collective_compute(
                    "AllReduce", Alu.add,
                    replica_groups=GROUPS,
                    ins=[arin[qc].opt()], outs=[arout[qc].opt()],
                )

            # ---------------- phase E: tail on own 512 tokens ----------------
            # the AllReduce result is identical across the group; each core
            # reads it all (reusing xt's slot) and selects its own 512-token
            # slice with a runtime register offset (rofs, host-supplied).
            rsb = big.tile([128, NKT, T], bf16, tag="xt", name="rsb")
            for q in range(NQC):
                # gpsimd (SWDGE) so these collective-gated reads never stall
                # the sync queue that carries the arin bounce writes; per-k
                # so the x2 adds pipeline with the last chunk's read
                for k in range(NKT):
                    nc.gpsimd.dma_start(
                        out=rsb[:, k, q * QCW:(q + 1) * QCW],
                        in_=arout[q][k, :, :])
            if DEBUG:
                nc.sync.dma_start(out=dbg_qk[:, :, :], in_=qkT[:, :, :])
                nc.sync.dma_start(out=dbg_v[:, :, :, :], in_=v_sb[:, :, :, :])
                nc.sync.dma_start(out=dbg_at[:, :, :], in_=attnT[:, :, :])
                nc.sync.dma_start(out=dbg_rs[:, :, :], in_=rsb[:, :, :])
            rreg = nc.vector.alloc_register("rofs_reg")
            nc.vector.reg_load(rreg, rofs[0:1, 0:1])
            roff = nc.vector.snap(rreg, donate=True, min_val=0,
                                  max_val=T - TPC)
            x2 = big.tile([128, NKT, TPC], f32, tag="x2")
            x2b = big.tile([128, NKT, TPC], bf16, tag="h2", name="x2b")
            # rmsnorm2 stats (same ln/exp trick); the x2b half-0 casts are
            # emitted first so rotation pass 0 (which needs only half 0)
            # starts asap, then stats follow
            lnms2 = small.tile([1, TPC], f32, tag="lnms2")
            rstd2 = small.tile([1, TPC], bf16, tag="rstd2")
            rstd2B = small.tile([128, TPC], bf16, tag="rstd2B")
            bp2 = pso.tile([128, 2, QCW], f32, tag="out", name="bp2")
            h0 = slice(0, TPC // 2)
            h1 = slice(TPC // 2, TPC)
            for k in range(NKT):
                xo_k = med.tile([128, TPC], f32, tag="xo")
                nc.sync.dma_start(out=xo_k[:, :], in_=xto[k, :, :])
                nc.vector.tensor_add(out=x2[:, k, :], in0=xo_k[:, :],
                                     in1=rsb[:, k, bass.ds(roff, TPC)])
                nc.vector.tensor_copy(out=x2b[:, k, h0], in_=x2[:, k, h0])
            ssq2 = pso.tile([1, TPC], f32, tag="out", name="ssq2")
            for k in range(NKT):
                nc.vector.tensor_copy(out=x2b[:, k, h1], in_=x2[:, k, h1])
                sq2 = med.tile([128, TPC], bf16, tag="sq2")
                nc.scalar.activation(out=sq2[:, :], in_=x2[:, k, :],
                                     func=AF.Square)
                nc.tensor.matmul(ssq2[:, :], ones_b[:, :], sq2[:, :],
                                 start=(k == 0), stop=(k == NKT - 1))
            nc.scalar.activation(out=lnms2[:, :], in_=ssq2[:, :],
                                 func=AF.Ln, scale=1.0 / D, bias=eps_t[:, :])
            nc.scalar.activation(out=rstd2[:, :], in_=lnms2[:, :],
                                 func=AF.Exp, scale=-0.5)
            nc.tensor.matmul(bp2[:, 0, :], onr_b[:, :], rstd2[:, :],
                             start=True, stop=True)
            nc.vector.tensor_copy(out=rstd2B[:, :], in_=bp2[:, 0, :])
            # geff2 is folded into G_0 on the host (plus a beta@G_0 bias
            # shift) and rstd2 commutes through the pass-0 matmul as a
            # per-token scalar: pass 0 runs on raw bf16 x2 (cast above, no
            # wait on the norm2 stats); rstd2 is applied at PSUM evacuation.

            # rotation passes (gate folded into G; bias separate).  G for
            # pass 0/2 lands in the freed wq slots, pass 1 in the freed qkT
            # slot; all three loads are emitted early so they prefetch
            # behind the AllReduce chain.
            gh01 = big.tile([128, 4, D], bf16, tag="wq", bufs=2, name="gh01")
            nc.sync.dma_start(out=gh01[:, :, :],
                              in_=gmat[0, 0:4, :, :].rearrange("k p t -> p k t"))
            gh00 = big.tile([128, 4, D], bf16, tag="wq", bufs=2, name="gh00")
            nc.sync.dma_start(out=gh00[:, :, :],
                              in_=gmat[0, 4:8, :, :].rearrange("k p t -> p k t"))
            gh1 = big.tile([128, NKT, D], bf16, tag="qkT", name="gh1")
            nc.sync.dma_start(out=gh1[:, :, :],
                              in_=gmat[1, :, :, :].rearrange("k p t -> p k t"))
            gslices = {
                0: [gh01[:, :, :], gh00[:, :, :]],
                1: [gh1[:, 0:4, :], gh1[:, 4:8, :]],
            }
            rb0 = big.tile([128, NKT, TPC], bf16, tag="rb0")
            rb1 = big.tile([128, NKT, TPC], bf16, tag="rb1")
            cur = x2b
            for p3 in range(NPASS):
                last = p3 == NPASS - 1
                nxt = None if last else (rb0 if p3 == 0 else rb1)
                if p3 < 2:
                    gh = gslices[p3]
                else:
                    gh = []
                    for half in range(2):
                        gt_ = big.tile([128, 4, D], bf16, tag="wq", bufs=2,
                                       name=f"gh2{half}")
                        nc.sync.dma_start(
                            out=gt_[:, :, :],
                            in_=gmat[2, 4 * half:4 * half + 4, :, :]
                            .rearrange("k p t -> p k t"))
                        gh.append(gt_[:, :, :])
                # token-halved so consecutive passes pipeline: pass p+1 on
                # half 0 starts once pass p finished half 0
                for th in range(2):
                    tsl = slice(th * (TPC // 2), (th + 1) * (TPC // 2))
                    for m in range(NKT):
                        rp = ps.tile([128, 2, QCW], f32, tag="mm",
                                     name=f"r{p3}{m}{th}")
                        for k in range(NKT):
                            nc.tensor.matmul(
                                rp[:, 0, 0:TPC // 2],
                                gh[k // 4][:, k % 4,
                                           m * 128:(m + 1) * 128],
                                cur[:, k, tsl],
                                start=(k == 0), stop=(k == NKT - 1))
                        if p3 == 0:
                            # apply the commuted rstd2 before the silu
                            rn = med.tile([128, TPC // 2], bf16, tag="rn")
                            nc.vector.tensor_mul(out=rn[:, :],
                                                 in0=rp[:, 0, 0:TPC // 2],
                                                 in1=rstd2B[:, tsl])
                            nc.scalar.activation(
                                out=nxt[:, m, tsl], in_=rn[:, :],
                                func=AF.Silu, bias=br_t[:, m, p3:p3 + 1])
                        elif not last:
                            nc.scalar.activation(
                                out=nxt[:, m, tsl], in_=rp[:, 0, 0:TPC // 2],
                                func=AF.Silu, bias=br_t[:, m, p3:p3 + 1])
                        else:
                            # final: y_m = (x2_m - h2_m) + silu(rp); the
                            # subtraction was folded into x2 during pass 1
                            rl = med.tile([128, TPC // 2], f32, tag="rl")
                            nc.scalar.activation(
                                out=rl[:, :], in_=rp[:, 0, 0:TPC // 2],
                                func=AF.Silu, bias=br_t[:, m, p3:p3 + 1])
                            nc.vector.tensor_add(out=x2[:, m, tsl],
                                                 in0=x2[:, m, tsl],
                                                 in1=rl[:, :])
                            if m == 3:
                                # first m-half output overlaps the rest
                                nc.sync.dma_start(
                                    out=yt[0:4, :, tsl]
                                    .rearrange("k p t -> p k t"),
                                    in_=x2[:, 0:4, tsl])
                    if last:
                        nc.sync.dma_start(
                            out=yt[4:8, :, tsl].rearrange("k p t -> p k t"),
                            in_=x2[:, 4:8, tsl])
                if p3 == 0:
                    # h2 and the (x2 - h2) fold, overlapped with pass 1
                    for k in range(NKT):
                        h2k = med.tile([128, TPC], bf16, tag="h2k", bufs=2,
                                       name=f"h2k{k}")
                        nc.vector.tensor_mul(out=h2k[:, :],
                                             in0=x2b[:, k, :],
                                             in1=rstd2B[:, :])
                        nc.vector.tensor_scalar(
                            out=h2k[:, :], in0=h2k[:, :],
                            scalar1=g2_t[:, k:k + 1],
                            scalar2=be_t[:, k:k + 1],
                            op0=Alu.mult, op1=Alu.add)
                        nc.vector.tensor_sub(out=x2[:, k, :],
                                             in0=x2[:, k, :],
                                             in1=h2k[:, :])
                cur = nxt
    return nc


def _prep_inputs(x, scale_gamma, scale_beta, qkv_w, o_w, norm1_w, norm2_w,
                 angles, gate, bias, pi, pj):
    import ml_dtypes
    bf = ml_dtypes.bfloat16
    x = np.asarray(x, np.float32)
    gmats = _giv_mats(np.asarray(angles), np.asarray(pi), np.asarray(pj),
                      np.asarray(gate))
    g1 = (np.asarray(norm1_w) * np.asarray(scale_gamma)).astype(np.float32)
    g2 = (np.asarray(norm2_w) * np.asarray(scale_gamma)).astype(np.float32)
    beta = np.asarray(scale_beta, np.float32)
    # fold geff2 into G_0 (rotation pass 0 runs on x2n = x2*rstd2); the
    # beta part of h2 projects into the pass-0 bias
    bias = np.asarray(bias, np.float32).copy()
    bias[0] = bias[0] + beta @ gmats[0]
    gmats[0] = g2[:, None] * gmats[0]
    gm = np.stack([g.reshape(NKT, 128, D) for g in gmats]).astype(bf)
    g1_t = np.ascontiguousarray(g1.reshape(NKT, 128).T)
    g2_t = np.ascontiguousarray(g2.reshape(NKT, 128).T)
    be_t = np.ascontiguousarray(beta.reshape(NKT, 128).T)
    br = bias.reshape(NPASS, NKT, 128)
    br_t = np.ascontiguousarray(np.transpose(br, (2, 1, 0)))
    tri = np.tril(np.ones((128, 128), np.float32)).T  # [kl, ql] kl<=ql
    tri2 = np.ascontiguousarray(
        np.broadcast_to(tri[:, None, :], (128, 2, 128))).astype(bf)
    qkv_wT = np.asarray(qkv_w, np.float32).T  # [D, 3D]
    o_wT = np.asarray(o_w, np.float32).T      # [D(d), D(j)]

    in_maps = []
    for c in range(NCORES):
        b, r = divmod(c, R)
        xT = np.ascontiguousarray(x[b].T)          # [D, T]
        hsel = np.r_[256 * r:256 * r + 256]
        wsel = np.concatenate([qkv_wT[:, hsel], qkv_wT[:, 1024 + hsel],
                               qkv_wT[:, 2048 + hsel]], axis=1)  # [D, 768]
        # fold geff1 into the weight rows; beta projects to a const vector
        wfold = wsel * g1[:, None]
        cvec = beta @ wsel                          # [768]
        cqk_m = np.ascontiguousarray(
            cvec[:512].reshape(4, 128).T).astype(np.float32)   # [128, 4]
        cvb_m = np.ascontiguousarray(
            np.broadcast_to(cvec[512:768], (128, 256))).astype(bf)
        m = dict(
            xtb=xT.reshape(NKT, 128, T).astype(bf),
            xto=np.ascontiguousarray(
                xT[:, r * TPC:(r + 1) * TPC]).reshape(NKT, 128, TPC),
            wqkv=np.ascontiguousarray(wfold).reshape(NKT, 128, 768).astype(bf),
            wow=np.ascontiguousarray(
                o_wT[256 * r:256 * r + 256]).reshape(2, 128, D).astype(bf),
            gmat=gm,
            geff1=g1_t, geff2=g2_t, betav=be_t, biasr=br_t,
            trim=tri2,
            cqk=cqk_m, cvb=cvb_m,
            rofs=np.array([[r * TPC, r * TPC + TPC // 2]], np.uint32),
        )
        in_maps.append(m)
    return in_maps


_NC_CACHE = [None]


def _device_run(in_maps):
    from concourse import bass_utils
    import concourse.mybir as mybir

    if _NC_CACHE[0] is None:
        nc = _build_nc()
        _split_multi_waits(nc, mybir)
        _NC_CACHE[0] = nc
    nc = _NC_CACHE[0]
    res = bass_utils.run_bass_kernel_spmd(nc, in_maps,
                                          core_ids=list(range(NCORES)))
    y = np.empty((B, T, D), np.float32)
    for c in range(NCORES):
        b, r = divmod(c, R)
        yT = res.results[c]["yt"].reshape(D, TPC)
        y[b, r * TPC:(r + 1) * TPC] = yT.T
    return y


def kernel(x, scale_gamma, scale_beta, qkv_w, o_w, norm1_w, norm2_w,
           angles, gate, bias, pi, pj):
    args = dict(x=x, scale_gamma=scale_gamma, scale_beta=scale_beta,
                qkv_w=qkv_w, o_w=o_w, norm1_w=norm1_w, norm2_w=norm2_w,
                angles=angles, gate=gate, bias=bias, pi=pi, pj=pj)
    try:
        in_maps = _prep_inputs(**args)
        return _device_run(in_maps)
    except Exception as e:
        print(f"device path failed ({type(e).__name__}: {e}); "
              "using host fallback", file=sys.stderr)
        import traceback
        traceback.print_exc()
        return _host_full(**args)
